# revision 46
# baseline (speedup 1.0000x reference)
"""Multi-head causal attention on 8 TRN2 NeuronCores.

Problem: x[4,2048,1024] @ Wqkv.T -> 16-head causal attention -> @ Wout.T.

Sharding: core c handles batch b=c//2, head-group g=c%2 (8 heads of 64).
Each core computes qkv for its (batch, head-group) slice, causal attention,
and a partial out-projection over its 512 columns of Wout's input dim.
Host sums the two partials per batch (the all-reduce of the hint).

Per-core layouts (host pre-transposes so every matmul contraction dim lands
on SBUF partitions):
  xT   [1024 d, 2048 t]      wqkT [1024 d, 1024 (q|k)e]
  wvT  [1024 d,  512 e]      woT  [ 512 e, 1024 f]
Q/K/S stay fp32r; P (post-exp) and V are bf16 (err budget 2e-2 allows it).

Schedule: the TRN2 PE clock ramps (1.2 GHz until ~3us of continuous busy,
then 2.4 GHz), so any PE idle gap doubles matmul time.  The attention inner
loop is a depth-3 pipeline over 128-key j-blocks:
    step s:  S(s) matmuls | exp(s-1) on ACT | mask(s-1) on gpsimd | AV(s-3)
so the scalar-engine exp (~1.1us/block vs ~0.85us of PE work) is never on
the PE critical path.  The ACT deficit is covered by interleaving next-chunk
QKV-production and prev-chunk out-projection matmuls as per-step PE filler,
assigned to windows/pairs so tile-pool slot rotation never waits on a
later-emitted reader:
  window tc:  pair0 <- out-proj(tc-1); pair p -> qk/v group p-1 of chunk
  tc+1; group 3 after pair 3.  qk(3) of chunk 3 runs inside window 3
  (pairs 0-2 don't read kt[3]/qt[3]); out-proj(3) is the epilogue.
Diagonal j-blocks only stream/exp query columns >= 128r (clamped to N>=256
for fp32r); affine_select zeroes the masked+stale region afterwards.
"""

import sys

sys.path.insert(0, "/opt/trn_rl_repo")

import numpy as np

B, T, D, H = 4, 2048, 1024, 16
E = 512  # per-core head width (8 heads x 64)
ND = 8  # d chunks of 128
NTC = 4  # t chunks of 512
SCALE = 0.125  # 1/sqrt(64)
Q0R = [0, 128, 256, 256]  # first live query col per diag sub-block r
LAG = 3  # AV trails S by LAG j-blocks

_NC_CACHE = {}


def build():
    if "nc" in _NC_CACHE:
        return _NC_CACHE["nc"]
    import concourse.bacc as bacc
    import concourse.mybir as mybir
    import concourse.tile as tile

    F32 = mybir.dt.float32
    F32R = mybir.dt.float32r
    BF16 = mybir.dt.bfloat16
    EXP = mybir.ActivationFunctionType.Exp

    nc = bacc.Bacc("TRN2", target_bir_lowering=False, debug=False, num_devices=8)
    xT = nc.declare_dram_parameter("xT", [D, T], BF16, isOutput=False)
    wqkT = nc.declare_dram_parameter("wqkT", [D, 2 * E], BF16, isOutput=False)
    wvT = nc.declare_dram_parameter("wvT", [D, E], BF16, isOutput=False)
    woT = nc.declare_dram_parameter("woT", [E, D], F32R, isOutput=False)
    z = nc.declare_dram_parameter("z", [T, D], F32, isOutput=True)
    dbg = {}
    if _NC_CACHE.get("debug"):
        for nm, shp in [
            ("dpt", [128, 512]), ("dya", [96, 512]), ("drca", [1, 512]),
            ("drba", [64, 512]), ("drbb", [64, 512]), ("dysb", [128, 512]),
            ("dytm", [64, 512]),
        ]:
            dbg[nm] = nc.declare_dram_parameter(nm, shp, F32, isOutput=True)

    with tile.TileContext(nc) as tc_:
        with (
            tc_.tile_pool(name="pw", bufs=8) as pw,
            tc_.tile_pool(name="pwo", bufs=4) as pwo,
            tc_.tile_pool(name="px", bufs=16) as px,
            tc_.tile_pool(name="pkt", bufs=4) as pkt,
            tc_.tile_pool(name="pqt", bufs=4) as pqt,
            tc_.tile_pool(name="pv", bufs=16) as pv,
            tc_.tile_pool(name="ppt", bufs=4) as ppt,
            tc_.tile_pool(name="pr", bufs=2) as pr,
            tc_.tile_pool(name="pysb", bufs=6) as pysb,
            tc_.tile_pool(name="pzsb", bufs=4) as pzsb,
            tc_.tile_pool(name="pst", bufs=2, space="PSUM") as pst,
            tc_.tile_pool(name="pyd", bufs=1, space="PSUM") as pyd,
            tc_.tile_pool(name="pfa", bufs=2, space="PSUM") as pfa,
        ):
            # ---- weights + first x chunks, striped across DMA queues and
            # ordered so the dc=0 QKV chain can start within ~5us: the
            # prologue is input-DMA-bound, so emission order is load order.
            def dma_striped(dst, src, nstripe):
                w = dst.shape[1]
                sw = w // nstripe
                for s_ in range(nstripe):
                    nc.sync.dma_start(
                        dst[:, s_ * sw : (s_ + 1) * sw],
                        src[:, s_ * sw : (s_ + 1) * sw],
                    )

            def emit_x_loads(tci):
                xs = []
                t0 = tci * 512
                for dc in range(ND):
                    t_ = px.tile([128, 512], BF16, tag="x", name="xs")
                    nc.sync.dma_start(
                        t_[:], xT[dc * 128 : (dc + 1) * 128, t0 : t0 + 512]
                    )
                    xs.append(t_)
                return xs

            wqk = []
            xs_p = []
            for dc in range(ND):
                t_ = pw.tile([128, 2 * E], BF16, tag="wqk")
                dma_striped(t_[:], wqkT[dc * 128 : (dc + 1) * 128, :], 4)
                wqk.append(t_)
                tx = px.tile([128, 512], BF16, tag="x", name="xs")
                dma_striped(tx[:], xT[dc * 128 : (dc + 1) * 128, 0:512], 2)
                xs_p.append(tx)
            wv = []
            for dc in range(ND):
                t_ = pw.tile([128, E], BF16, tag="wv")
                nc.sync.dma_start(t_[:], wvT[dc * 128 : (dc + 1) * 128, :])
                wv.append(t_)
            xs_by_tc = {0: xs_p, 1: emit_x_loads(1)}
            wo = []
            for m in range(4):
                t_ = pwo.tile([128, D], F32R, tag="wo")
                nc.sync.dma_start(t_[:], woT[m * 128 : (m + 1) * 128, :])
                wo.append(t_)

            # persistent K^T [e,t] tiles; pair m = heads 2m / 2m+1 at
            # partition rows 0:64 / 64:128
            kt = [
                pkt.tile([128, T], F32R, tag="kt", name=f"kt{i}")
                for i in range(4)
            ]
            vt = [None] * 16  # bf16 [V_h(64)|ones(32)] per head, per j-block
            qt_by_tc = {}  # (tc, m) -> qt tile
            ysb_hist = {}  # (tc, m) -> normalized y^T tile

            # ---- filler chains (lists of zero-arg closures, one PE/DVE op
            # each, executed by the window scheduler's filler cursor)

            def qk_chain(xs, g, tci):
                """g 0..3: Q chunk for pair g; 4..7: K chunk for pair g-4."""
                ops = []
                state = {}

                def mk_mm(dc):
                    def run():
                        if "acc" not in state:
                            state["acc"] = pfa.tile(
                                [128, 512], F32, tag="facc", name="qkacc"
                            )
                        nc.tensor.matmul(
                            state["acc"][:],
                            wqk[dc][:, g * 128 : (g + 1) * 128],
                            xs[dc][:],
                            start=(dc == 0),
                            stop=(dc == ND - 1),
                        )

                    return run

                ops += [mk_mm(dc) for dc in range(ND)]

                def fin():
                    acc = state["acc"]
                    if g < 4:
                        t_ = pqt.tile([128, 512], F32R, tag="qt", name="qt")
                        nc.vector.tensor_copy(t_[:], acc[:])
                        qt_by_tc[(tci, g)] = t_
                    else:
                        t0 = tci * 512
                        nc.vector.tensor_copy(
                            kt[g - 4][:, t0 : t0 + 512], acc[:]
                        )

                ops.append(fin)
                return ops

            def v_chain(xs, tci, ts):
                jb = 4 * tci + ts
                ops = []
                state = {}

                def mk_mm(dc):
                    def run():
                        if "acc" not in state:
                            state["acc"] = pfa.tile(
                                [128, 512], F32, tag="facc", name="vacc"
                            )
                        nc.tensor.matmul(
                            state["acc"][:],
                            xs[dc][:, ts * 128 : (ts + 1) * 128],
                            wv[dc][:],
                            start=(dc == 0),
                            stop=(dc == ND - 1),
                        )

                    return run

                ops += [mk_mm(dc) for dc in range(ND)]

                def fin():
                    acc = state["acc"]
                    t_ = pv.tile([128, 768], BF16, tag="v", name="vt")
                    t4 = t_[:].rearrange("p (hh c) -> p hh c", hh=8)
                    a4 = acc[:].rearrange("p (hh c) -> p hh c", hh=8)
                    nc.vector.tensor_copy(t4[:, :, 0:64], a4[:])
                    nc.vector.memset(t4[:, :, 64:96], 1.0)
                    vt[jb] = t_

                ops.append(fin)
                return ops

            def outproj_chain(tcp, ib, fh):
                ops = []
                state = {}

                def mk_mm(m):
                    def run():
                        if "zp" not in state:
                            state["zp"] = pfa.tile(
                                [128, 512], F32, tag="facc", name="zp"
                            )
                        nc.tensor.matmul(
                            state["zp"][:],
                            ysb_hist[(tcp, m)][:, ib * 128 : (ib + 1) * 128],
                            wo[m][:, fh * 512 : fh * 512 + 512],
                            start=(m == 0),
                            stop=(m == 3),
                        )

                    return run

                ops += [mk_mm(m) for m in range(4)]

                def fin():
                    zsb = pzsb.tile([128, 512], F32, tag="zsb")
                    nc.vector.tensor_copy(zsb[:], state["zp"][:])
                    row = (4 * tcp + ib) * 128
                    ns_ = 4 if tcp == NTC - 1 else 2
                    sw_ = 512 // ns_
                    for s_ in range(ns_):
                        nc.sync.dma_start(
                            z[
                                row : row + 128,
                                fh * 512 + s_ * sw_ : fh * 512 + (s_ + 1) * sw_,
                            ],
                            zsb[:, s_ * sw_ : (s_ + 1) * sw_],
                        )

                ops.append(fin)
                return ops

            def outproj_chains(tcp):
                return [
                    outproj_chain(tcp, ib, fh)
                    for ib in range(4)
                    for fh in range(2)
                ]

            # ---- attention pieces
            def emit_S(tci, m, jb, qtm):
                st = pst.tile([128, 1024], F32, tag="st", name="st")
                r = jb - 4 * tci
                q0 = Q0R[r] if r >= 0 else 0
                for h in range(2):
                    nc.tensor.matmul(
                        st[:, h * 512 + q0 : (h + 1) * 512],
                        kt[m][
                            h * 64 : h * 64 + 64, jb * 128 : (jb + 1) * 128
                        ],
                        qtm[h * 64 : h * 64 + 64, q0:512],
                        start=True,
                        stop=True,
                    )
                return st

            def emit_exp(tci, jb, st):
                pt = ppt.tile([128, 1024], BF16, tag="pt", name="pt")
                r = jb - 4 * tci
                q0 = Q0R[r] if r >= 0 else 0
                if q0 > 0:
                    s3 = st[:].rearrange("p (h q) -> p h q", h=2)
                    p3 = pt[:].rearrange("p (h q) -> p h q", h=2)
                    nc.scalar.activation(
                        p3[:, :, q0:512], s3[:, :, q0:512], EXP, scale=SCALE
                    )
                    # cols < q0 are entirely masked and never exp'd
                    nc.vector.memset(p3[:, :, 0:q0], 0.0)
                else:
                    nc.scalar.activation(pt[:], st[:], EXP, scale=SCALE)
                if r >= 0:
                    # zero masked region in the live cols: keep where
                    # (q' + q0) - k - 128 r >= 0
                    p3 = pt[:].rearrange("p (h q) -> p h q", h=2)
                    nc.gpsimd.affine_select(
                        out=p3[:, :, q0:512],
                        in_=p3[:, :, q0:512],
                        compare_op=mybir.AluOpType.is_ge,
                        fill=0.0,
                        base=q0 - 128 * r,
                        pattern=[[0, 2], [1, 512 - q0]],
                        channel_multiplier=-1,
                    )
                return pt

            def emit_AV(m, jb, pt, ya, yb, first, last):
                nc.tensor.matmul(
                    ya[:],
                    vt[jb][:, m * 192 : m * 192 + 96],
                    pt[:, 0:512],
                    start=first,
                    stop=last,
                )
                nc.tensor.matmul(
                    yb[:],
                    vt[jb][:, m * 192 + 96 : m * 192 + 192],
                    pt[:, 512:1024],
                    start=first,
                    stop=last,
                )

            def emit_normalize(tci, m, ya, yb):
                if dbg and tci == 0 and m == 0:
                    dya_sb = pzsb.tile([128, 512], F32, tag="zsb")
                    nc.vector.tensor_copy(dya_sb[0:96, :], ya[:])
                    nc.sync.dma_start(dbg["dya"][:], dya_sb[0:96, :])
                # ya/yb rows 64:96 hold the softmax denominator (ones
                # columns of vt); reciprocal runs in-place at partition 64,
                # gpsimd broadcasts it down to rows 0:64.
                # critical path after the last AV: copies (DVE+gpsimd in
                # parallel) -> rc0 gather (gen on the idle gpsimd queue so
                # it can't head-of-line-block the sync DMA queue) -> recip
                # -> one wide broadcast -> muls (DVE+gpsimd) -> shuffle
                # (DVE partition move; no DMA anywhere near the hot path)
                rca = pr.tile([128, 1024], F32, tag="rca", bufs=1)
                nc.vector.tensor_copy(rca[64:65, 0:512], ya[64:65, :])
                nc.scalar.copy(rca[64:65, 512:1024], yb[64:65, :])
                # partition-move head B's raw y to rows 64:128 (off the
                # critical path; the mul below does the fp32r rounding)
                ytmp = pr.tile([128, 512], F32, tag="ytmp", bufs=1)
                nc.vector.stream_shuffle(
                    ytmp[64:128, :], yb[0:64, :], mask=list(range(32))
                )
                rc0 = pr.tile([1, 1024], F32, tag="rc0", bufs=1)
                nc.gpsimd.dma_start(rc0[0:1, :], rca[64:65, :])
                rcv = pr.tile([1, 1024], F32, tag="rcv", bufs=1)
                nc.vector.reciprocal_approx_fast(rcv[0:1, :], rc0[0:1, :])
                rb = pr.tile([128, 1024], F32, tag="rb", bufs=1)
                nc.gpsimd.partition_broadcast(rb[:, :], rcv[0:1, :])
                ysb = pysb.tile([128, 512], F32R, tag="ysb", name="ysb")
                nc.vector.tensor_mul(
                    ysb[0:64, :], ya[0:64, :], rb[0:64, 0:512]
                )
                nc.vector.tensor_mul(
                    ysb[64:128, :], ytmp[64:128, :], rb[64:128, 512:1024]
                )
                if dbg and tci == 0 and m == 0:
                    nc.sync.dma_start(dbg["drca"][:], rcv[0:1, 0:512])
                    nc.sync.dma_start(dbg["drba"][:], rb[0:64, 0:512])
                    nc.sync.dma_start(dbg["drbb"][:], rb[64:128, 512:1024])
                    nc.sync.dma_start(dbg["dytm"][:], ytmp[64:128, :])
                    nc.sync.dma_start(dbg["dysb"][:], ysb[:].bitcast(F32))
                ysb_hist[(tci, m)] = ysb

            # ---- prologue: chunk 0 QKV, dense
            xs_cur = xs_by_tc[0]
            for g in range(8):
                for op in qk_chain(xs_cur, g, 0):
                    op()
            for ts in range(4):
                for op in v_chain(xs_cur, 0, ts):
                    op()

            # ---- windows
            for tci in range(NTC):
                njb = 4 * tci + 4
                pairfill = [[], [], [], []]
                endfill = []
                if tci + 2 < NTC:
                    xs_by_tc[tci + 2] = emit_x_loads(tci + 2)
                if tci >= 1:
                    # spread prev chunk's out-projection across all pairs
                    for i, ch in enumerate(outproj_chains(tci - 1)):
                        pairfill[i % 4] += ch
                if tci + 1 < NTC:
                    xs_next = xs_by_tc[tci + 1]
                    for g in range(3):
                        pairfill[g + 1] += qk_chain(xs_next, g, tci + 1)
                        pairfill[g + 1] += qk_chain(xs_next, g + 4, tci + 1)
                        pairfill[g + 1] += v_chain(xs_next, tci + 1, g)
                    if tci < 2:
                        endfill += qk_chain(xs_next, 3, tci + 1)
                        endfill += qk_chain(xs_next, 7, tci + 1)
                        endfill += v_chain(xs_next, tci + 1, 3)
                    else:
                        # chunk 3's pair-3 qk runs inside window 3 (whose
                        # pairs 0-2 don't touch kt[3]/qt[3]) to balance
                        # window 3's PE load against its ACT load
                        endfill += v_chain(xs_next, tci + 1, 3)
                        _w3qk = qk_chain(xs_next, 3, tci + 1) + qk_chain(
                            xs_next, 7, tci + 1
                        )
                else:
                    pairfill[1] = _w3qk + pairfill[1]

                for m in range(4):
                    qtm = qt_by_tc[(tci, m)]
                    ya = pyd.tile([96, 512], F32, tag="ya", bufs=1)
                    yb = pyd.tile([96, 512], F32, tag="yb", bufs=1)
                    fl = pairfill[m]
                    fcur = 0
                    nsteps = njb + LAG
                    ring_st = [None] * 2
                    ring_pt = [None] * 4
                    for s in range(nsteps):
                        if s < njb:
                            ring_st[s % 2] = emit_S(tci, m, s, qtm)
                        if 1 <= s <= njb:
                            jb = s - 1
                            ring_pt[jb % 4] = emit_exp(
                                tci, jb, ring_st[jb % 2]
                            )
                            if dbg and tci == 0 and m == 0 and jb == 0:
                                nc.sync.dma_start(
                                    dbg["dpt"][:],
                                    ring_pt[0][:].bitcast(F32),
                                )
                        if LAG <= s < njb + LAG:
                            jb = s - LAG
                            emit_AV(
                                m,
                                jb,
                                ring_pt[jb % 4],
                                ya,
                                yb,
                                jb == 0,
                                jb == njb - 1,
                            )
                        target = len(fl) * (s + 1) // nsteps
                        while fcur < target:
                            fl[fcur]()
                            fcur += 1
                    emit_normalize(tci, m, ya, yb)
                for op in endfill:
                    op()

            # ---- epilogue: last chunk's out-projection
            for ch in outproj_chains(NTC - 1):
                for op in ch:
                    op()

    nc.finalize()
    _NC_CACHE["nc"] = nc
    return nc


def _in_maps(x, Wqkv, Wout):
    import ml_dtypes

    bf16 = ml_dtypes.bfloat16
    x = np.ascontiguousarray(np.asarray(x, dtype=np.float32))
    Wqkv = np.ascontiguousarray(np.asarray(Wqkv, dtype=np.float32))
    Wout = np.ascontiguousarray(np.asarray(Wout, dtype=np.float32))
    xTs = [np.ascontiguousarray(x[b].T.astype(bf16)) for b in range(B)]
    maps = []
    for c in range(8):
        b, g = divmod(c, 2)
        qrows = Wqkv[E * g : E * g + E]
        krows = Wqkv[D + E * g : D + E * g + E]
        vrows = Wqkv[2 * D + E * g : 2 * D + E * g + E]
        maps.append(
            {
                "xT": xTs[b],
                "wqkT": np.ascontiguousarray(
                    np.concatenate([qrows, krows], axis=0).T.astype(bf16)
                ),
                "wvT": np.ascontiguousarray(vrows.T.astype(bf16)),
                "woT": np.ascontiguousarray(Wout[:, E * g : E * g + E].T),
            }
        )
    return maps


def _run(x, Wqkv, Wout, trace=False):
    from concourse.bass_utils import run_bass_kernel_spmd

    nc = build()
    res = run_bass_kernel_spmd(
        nc, _in_maps(x, Wqkv, Wout), core_ids=list(range(8)), trace=trace
    )
    out = np.empty((B, T, D), dtype=np.float32)
    for b in range(B):
        out[b] = res.results[2 * b]["z"] + res.results[2 * b + 1]["z"]
    return out, res


def kernel(x, Wqkv, Wout):
    out, _ = _run(x, Wqkv, Wout, trace=False)
    return out


# revision 47
# speedup vs baseline: 1.0174x; 1.0174x over previous
"""Multi-head causal attention on 8 TRN2 NeuronCores.

Problem: x[4,2048,1024] @ Wqkv.T -> 16-head causal attention -> @ Wout.T.

Sharding: core c handles batch b=c//2, head-group g=c%2 (8 heads of 64).
Each core computes qkv for its (batch, head-group) slice, causal attention,
and a partial out-projection over its 512 columns of Wout's input dim.
Host sums the two partials per batch (the all-reduce of the hint).

Per-core layouts (host pre-transposes so every matmul contraction dim lands
on SBUF partitions):
  xT   [1024 d, 2048 t]      wqkT [1024 d, 1024 (q|k)e]
  wvT  [1024 d,  512 e]      woT  [ 512 e, 1024 f]
Q/K/S stay fp32r; P (post-exp) and V are bf16 (err budget 2e-2 allows it).

Schedule: the TRN2 PE clock ramps (1.2 GHz until ~3us of continuous busy,
then 2.4 GHz), so any PE idle gap doubles matmul time.  The attention inner
loop is a depth-3 pipeline over 128-key j-blocks:
    step s:  S(s) matmuls | exp(s-1) on ACT | mask(s-1) on gpsimd | AV(s-3)
so the scalar-engine exp (~1.1us/block vs ~0.85us of PE work) is never on
the PE critical path.  The ACT deficit is covered by interleaving next-chunk
QKV-production and prev-chunk out-projection matmuls as per-step PE filler,
assigned to windows/pairs so tile-pool slot rotation never waits on a
later-emitted reader:
  window tc:  pair0 <- out-proj(tc-1); pair p -> qk/v group p-1 of chunk
  tc+1; group 3 after pair 3.  qk(3) of chunk 3 runs inside window 3
  (pairs 0-2 don't read kt[3]/qt[3]); out-proj(3) is the epilogue.
Diagonal j-blocks only stream/exp query columns >= 128r (clamped to N>=256
for fp32r); affine_select zeroes the masked+stale region afterwards.
"""

import sys

sys.path.insert(0, "/opt/trn_rl_repo")

import numpy as np

B, T, D, H = 4, 2048, 1024, 16
E = 512  # per-core head width (8 heads x 64)
ND = 8  # d chunks of 128
NTC = 4  # t chunks of 512
SCALE = 0.125  # 1/sqrt(64)
Q0R = [0, 128, 256, 256]  # first live query col per diag sub-block r
LAG = 3  # AV trails S by LAG j-blocks

_NC_CACHE = {}


def build():
    if "nc" in _NC_CACHE:
        return _NC_CACHE["nc"]
    import concourse.bacc as bacc
    import concourse.mybir as mybir
    import concourse.tile as tile

    F32 = mybir.dt.float32
    F32R = mybir.dt.float32r
    BF16 = mybir.dt.bfloat16
    EXP = mybir.ActivationFunctionType.Exp

    nc = bacc.Bacc("TRN2", target_bir_lowering=False, debug=False, num_devices=8)
    xT = nc.declare_dram_parameter("xT", [D, T], BF16, isOutput=False)
    wqkT = nc.declare_dram_parameter("wqkT", [D, 2 * E], BF16, isOutput=False)
    wvT = nc.declare_dram_parameter("wvT", [D, E], BF16, isOutput=False)
    woT = nc.declare_dram_parameter("woT", [E, D], F32R, isOutput=False)
    z = nc.declare_dram_parameter("z", [T, D], F32, isOutput=True)
    dbg = {}
    if _NC_CACHE.get("debug"):
        for nm, shp in [
            ("dpt", [128, 512]), ("dya", [96, 512]), ("drca", [1, 512]),
            ("drba", [64, 512]), ("drbb", [64, 512]), ("dysb", [128, 512]),
            ("dytm", [64, 512]),
        ]:
            dbg[nm] = nc.declare_dram_parameter(nm, shp, F32, isOutput=True)

    with tile.TileContext(nc) as tc_:
        with (
            tc_.tile_pool(name="pw", bufs=8) as pw,
            tc_.tile_pool(name="pwo", bufs=4) as pwo,
            tc_.tile_pool(name="px", bufs=16) as px,
            tc_.tile_pool(name="pkt", bufs=4) as pkt,
            tc_.tile_pool(name="pqt", bufs=4) as pqt,
            tc_.tile_pool(name="pv", bufs=16) as pv,
            tc_.tile_pool(name="ppt", bufs=4) as ppt,
            tc_.tile_pool(name="pr", bufs=2) as pr,
            tc_.tile_pool(name="pysb", bufs=10) as pysb,
            tc_.tile_pool(name="pzsb", bufs=4) as pzsb,
            tc_.tile_pool(name="pst", bufs=2, space="PSUM") as pst,
            tc_.tile_pool(name="pyd", bufs=1, space="PSUM") as pyd,
            tc_.tile_pool(name="pfa", bufs=2, space="PSUM") as pfa,
        ):
            # ---- weights + first x chunks, striped across DMA queues and
            # ordered so the dc=0 QKV chain can start within ~5us: the
            # prologue is input-DMA-bound, so emission order is load order.
            def dma_striped(dst, src, nstripe):
                w = dst.shape[1]
                sw = w // nstripe
                for s_ in range(nstripe):
                    nc.sync.dma_start(
                        dst[:, s_ * sw : (s_ + 1) * sw],
                        src[:, s_ * sw : (s_ + 1) * sw],
                    )

            def emit_x_loads(tci):
                xs = []
                t0 = tci * 512
                for dc in range(ND):
                    t_ = px.tile([128, 512], BF16, tag="x", name="xs")
                    nc.sync.dma_start(
                        t_[:], xT[dc * 128 : (dc + 1) * 128, t0 : t0 + 512]
                    )
                    xs.append(t_)
                return xs

            wqk = []
            xs_p = []
            for dc in range(ND):
                t_ = pw.tile([128, 2 * E], BF16, tag="wqk")
                dma_striped(t_[:], wqkT[dc * 128 : (dc + 1) * 128, :], 4)
                wqk.append(t_)
                tx = px.tile([128, 512], BF16, tag="x", name="xs")
                dma_striped(tx[:], xT[dc * 128 : (dc + 1) * 128, 0:512], 2)
                xs_p.append(tx)
            wv = []
            for dc in range(ND):
                t_ = pw.tile([128, E], BF16, tag="wv")
                nc.sync.dma_start(t_[:], wvT[dc * 128 : (dc + 1) * 128, :])
                wv.append(t_)
            xs_by_tc = {0: xs_p, 1: emit_x_loads(1)}
            wo = []
            for m in range(4):
                t_ = pwo.tile([128, D], F32R, tag="wo")
                nc.sync.dma_start(t_[:], woT[m * 128 : (m + 1) * 128, :])
                wo.append(t_)

            # persistent K^T [e,t] tiles; pair m = heads 2m / 2m+1 at
            # partition rows 0:64 / 64:128
            kt = [
                pkt.tile([128, T], F32R, tag="kt", name=f"kt{i}")
                for i in range(4)
            ]
            vt = [None] * 16  # bf16 [V_h(64)|ones(32)] per head, per j-block
            qt_by_tc = {}  # (tc, m) -> qt tile
            ysb_hist = {}  # (tc, m) -> normalized y^T tile

            # ---- filler chains (lists of zero-arg closures, one PE/DVE op
            # each, executed by the window scheduler's filler cursor)

            def qk_chain(xs, g, tci):
                """g 0..3: Q chunk for pair g; 4..7: K chunk for pair g-4."""
                ops = []
                state = {}

                def mk_mm(dc):
                    def run():
                        if "acc" not in state:
                            state["acc"] = pfa.tile(
                                [128, 512], F32, tag="facc", name="qkacc"
                            )
                        nc.tensor.matmul(
                            state["acc"][:],
                            wqk[dc][:, g * 128 : (g + 1) * 128],
                            xs[dc][:],
                            start=(dc == 0),
                            stop=(dc == ND - 1),
                        )

                    return run

                ops += [mk_mm(dc) for dc in range(ND)]

                def fin():
                    acc = state["acc"]
                    if g < 4:
                        t_ = pqt.tile([128, 512], F32R, tag="qt", name="qt")
                        nc.vector.tensor_copy(t_[:], acc[:])
                        qt_by_tc[(tci, g)] = t_
                    else:
                        t0 = tci * 512
                        nc.vector.tensor_copy(
                            kt[g - 4][:, t0 : t0 + 512], acc[:]
                        )

                ops.append(fin)
                return ops

            def v_chain(xs, tci, ts):
                jb = 4 * tci + ts
                ops = []
                state = {}

                def mk_mm(dc):
                    def run():
                        if "acc" not in state:
                            state["acc"] = pfa.tile(
                                [128, 512], F32, tag="facc", name="vacc"
                            )
                        nc.tensor.matmul(
                            state["acc"][:],
                            xs[dc][:, ts * 128 : (ts + 1) * 128],
                            wv[dc][:],
                            start=(dc == 0),
                            stop=(dc == ND - 1),
                        )

                    return run

                ops += [mk_mm(dc) for dc in range(ND)]

                def fin():
                    acc = state["acc"]
                    t_ = pv.tile([128, 768], BF16, tag="v", name="vt")
                    t4 = t_[:].rearrange("p (hh c) -> p hh c", hh=8)
                    a4 = acc[:].rearrange("p (hh c) -> p hh c", hh=8)
                    nc.vector.tensor_copy(t4[:, :, 0:64], a4[:])
                    nc.vector.memset(t4[:, :, 64:96], 1.0)
                    vt[jb] = t_

                ops.append(fin)
                return ops

            def outproj_chain(tcp, ib, fh):
                ops = []
                state = {}

                def mk_mm(m):
                    def run():
                        if "zp" not in state:
                            state["zp"] = pfa.tile(
                                [128, 512], F32, tag="facc", name="zp"
                            )
                        nc.tensor.matmul(
                            state["zp"][:],
                            ysb_hist[(tcp, m)][:, ib * 128 : (ib + 1) * 128],
                            wo[m][:, fh * 512 : fh * 512 + 512],
                            start=(m == 0),
                            stop=(m == 3),
                        )

                    return run

                ops += [mk_mm(m) for m in range(4)]

                def fin():
                    zsb = pzsb.tile([128, 512], F32, tag="zsb")
                    nc.vector.tensor_copy(zsb[:], state["zp"][:])
                    row = (4 * tcp + ib) * 128
                    ns_ = 4 if tcp == NTC - 1 else 2
                    sw_ = 512 // ns_
                    for s_ in range(ns_):
                        nc.sync.dma_start(
                            z[
                                row : row + 128,
                                fh * 512 + s_ * sw_ : fh * 512 + (s_ + 1) * sw_,
                            ],
                            zsb[:, s_ * sw_ : (s_ + 1) * sw_],
                        )

                ops.append(fin)
                return ops

            def outproj_chains(tcp):
                return [
                    outproj_chain(tcp, ib, fh)
                    for ib in range(4)
                    for fh in range(2)
                ]

            # ---- attention pieces
            def emit_S(tci, m, jb, qtm):
                st = pst.tile([128, 1024], F32, tag="st", name="st")
                r = jb - 4 * tci
                q0 = Q0R[r] if r >= 0 else 0
                for h in range(2):
                    nc.tensor.matmul(
                        st[:, h * 512 + q0 : (h + 1) * 512],
                        kt[m][
                            h * 64 : h * 64 + 64, jb * 128 : (jb + 1) * 128
                        ],
                        qtm[h * 64 : h * 64 + 64, q0:512],
                        start=True,
                        stop=True,
                    )
                return st

            def emit_exp(tci, jb, st):
                pt = ppt.tile([128, 1024], BF16, tag="pt", name="pt")
                r = jb - 4 * tci
                q0 = Q0R[r] if r >= 0 else 0
                if q0 > 0:
                    s3 = st[:].rearrange("p (h q) -> p h q", h=2)
                    p3 = pt[:].rearrange("p (h q) -> p h q", h=2)
                    nc.scalar.activation(
                        p3[:, :, q0:512], s3[:, :, q0:512], EXP, scale=SCALE
                    )
                    # cols < q0 are entirely masked and never exp'd
                    nc.vector.memset(p3[:, :, 0:q0], 0.0)
                else:
                    nc.scalar.activation(pt[:], st[:], EXP, scale=SCALE)
                if r >= 0:
                    # zero masked region in the live cols: keep where
                    # (q' + q0) - k - 128 r >= 0
                    p3 = pt[:].rearrange("p (h q) -> p h q", h=2)
                    nc.gpsimd.affine_select(
                        out=p3[:, :, q0:512],
                        in_=p3[:, :, q0:512],
                        compare_op=mybir.AluOpType.is_ge,
                        fill=0.0,
                        base=q0 - 128 * r,
                        pattern=[[0, 2], [1, 512 - q0]],
                        channel_multiplier=-1,
                    )
                return pt

            def emit_AV(m, jb, pt, ya, yb, first, last):
                nc.tensor.matmul(
                    ya[:],
                    vt[jb][:, m * 192 : m * 192 + 96],
                    pt[:, 0:512],
                    start=first,
                    stop=last,
                )
                nc.tensor.matmul(
                    yb[:],
                    vt[jb][:, m * 192 + 96 : m * 192 + 192],
                    pt[:, 512:1024],
                    start=first,
                    stop=last,
                )

            def emit_normalize(tci, m, ya, yb):
                if dbg and tci == 0 and m == 0:
                    dya_sb = pzsb.tile([128, 512], F32, tag="zsb")
                    nc.vector.tensor_copy(dya_sb[0:96, :], ya[:])
                    nc.sync.dma_start(dbg["dya"][:], dya_sb[0:96, :])
                # ya/yb rows 64:96 hold the softmax denominator (ones
                # columns of vt); reciprocal runs in-place at partition 64,
                # gpsimd broadcasts it down to rows 0:64.
                # critical path after the last AV: copies (DVE+gpsimd in
                # parallel) -> rc0 gather (gen on the idle gpsimd queue so
                # it can't head-of-line-block the sync DMA queue) -> recip
                # -> one wide broadcast -> muls (DVE+gpsimd) -> shuffle
                # (DVE partition move; no DMA anywhere near the hot path)
                rca = pr.tile([128, 1024], F32, tag="rca", bufs=1)
                nc.vector.tensor_copy(rca[64:65, 0:512], ya[64:65, :])
                nc.vector.tensor_copy(rca[64:65, 512:1024], yb[64:65, :])
                # partition-move head B's raw y to rows 64:128 (off the
                # critical path; the mul below does the fp32r rounding)
                ytmp = pr.tile([128, 512], F32, tag="ytmp", bufs=1)
                nc.vector.stream_shuffle(
                    ytmp[64:128, :], yb[0:64, :], mask=list(range(32))
                )
                rc0 = pr.tile([1, 1024], F32, tag="rc0", bufs=1)
                nc.gpsimd.dma_start(rc0[0:1, :], rca[64:65, :])
                rcv = pr.tile([1, 1024], F32, tag="rcv", bufs=1)
                nc.vector.reciprocal_approx_fast(rcv[0:1, :], rc0[0:1, :])
                rb = pr.tile([128, 1024], F32, tag="rb", bufs=1)
                nc.gpsimd.partition_broadcast(rb[:, :], rcv[0:1, :])
                ysb = pysb.tile([128, 512], F32R, tag="ysb", name="ysb")
                nc.vector.tensor_mul(
                    ysb[0:64, :], ya[0:64, :], rb[0:64, 0:512]
                )
                nc.vector.tensor_mul(
                    ysb[64:128, :], ytmp[64:128, :], rb[64:128, 512:1024]
                )
                if dbg and tci == 0 and m == 0:
                    nc.sync.dma_start(dbg["drca"][:], rcv[0:1, 0:512])
                    nc.sync.dma_start(dbg["drba"][:], rb[0:64, 0:512])
                    nc.sync.dma_start(dbg["drbb"][:], rb[64:128, 512:1024])
                    nc.sync.dma_start(dbg["dytm"][:], ytmp[64:128, :])
                    nc.sync.dma_start(dbg["dysb"][:], ysb[:].bitcast(F32))
                ysb_hist[(tci, m)] = ysb

            # ---- prologue: chunk 0 QKV, dense
            xs_cur = xs_by_tc[0]
            for g in range(8):
                for op in qk_chain(xs_cur, g, 0):
                    op()
            for ts in range(4):
                for op in v_chain(xs_cur, 0, ts):
                    op()

            # ---- windows
            for tci in range(NTC):
                njb = 4 * tci + 4
                pairfill = [[], [], [], []]
                endfill = []
                if tci + 2 < NTC:
                    xs_by_tc[tci + 2] = emit_x_loads(tci + 2)
                if tci == 1:
                    for i, ch in enumerate(outproj_chains(0)):
                        pairfill[i % 4] += ch
                if tci + 1 < NTC:
                    xs_next = xs_by_tc[tci + 1]
                    if tci < 2:
                        for g in range(3):
                            pairfill[g + 1] += qk_chain(xs_next, g, tci + 1)
                            pairfill[g + 1] += qk_chain(
                                xs_next, g + 4, tci + 1
                            )
                            pairfill[g + 1] += v_chain(xs_next, tci + 1, g)
                        endfill += qk_chain(xs_next, 3, tci + 1)
                        endfill += qk_chain(xs_next, 7, tci + 1)
                        endfill += v_chain(xs_next, tci + 1, 3)
                    else:
                        # window 2: chunk 3's qkv minus its pair-3 qk, which
                        # runs inside window 3 (whose pairs 0-2 don't touch
                        # kt[3]/qt[3]); out-proj(1) and (2) also defer to
                        # window 3, whose ACT-paced steps need PE filler
                        pairfill[0] += v_chain(xs_next, tci + 1, 0)
                        pairfill[0] += v_chain(xs_next, tci + 1, 1)
                        for g in range(3):
                            pairfill[g + 1] += qk_chain(xs_next, g, tci + 1)
                            pairfill[g + 1] += qk_chain(
                                xs_next, g + 4, tci + 1
                            )
                        pairfill[3] += v_chain(xs_next, tci + 1, 2)
                        endfill += v_chain(xs_next, tci + 1, 3)
                        _w3qk = qk_chain(xs_next, 3, tci + 1) + qk_chain(
                            xs_next, 7, tci + 1
                        )
                else:
                    op1 = outproj_chains(1)
                    op2 = outproj_chains(2)
                    pairfill[0] = _w3qk + op1[0] + op1[1]
                    pairfill[1] = op1[2] + op1[3] + op1[4] + op1[5]
                    pairfill[2] = op1[6] + op1[7] + op2[0] + op2[1] + op2[2]
                    pairfill[3] = op2[3] + op2[4] + op2[5] + op2[6] + op2[7]

                for m in range(4):
                    qtm = qt_by_tc[(tci, m)]
                    ya = pyd.tile([96, 512], F32, tag="ya", bufs=1)
                    yb = pyd.tile([96, 512], F32, tag="yb", bufs=1)
                    fl = pairfill[m]
                    fcur = 0
                    nsteps = njb + LAG
                    ring_st = [None] * 2
                    ring_pt = [None] * 4
                    for s in range(nsteps):
                        if s < njb:
                            ring_st[s % 2] = emit_S(tci, m, s, qtm)
                        if 1 <= s <= njb:
                            jb = s - 1
                            ring_pt[jb % 4] = emit_exp(
                                tci, jb, ring_st[jb % 2]
                            )
                            if dbg and tci == 0 and m == 0 and jb == 0:
                                nc.sync.dma_start(
                                    dbg["dpt"][:],
                                    ring_pt[0][:].bitcast(F32),
                                )
                        if LAG <= s < njb + LAG:
                            jb = s - LAG
                            emit_AV(
                                m,
                                jb,
                                ring_pt[jb % 4],
                                ya,
                                yb,
                                jb == 0,
                                jb == njb - 1,
                            )
                        target = len(fl) * (s + 1) // nsteps
                        while fcur < target:
                            fl[fcur]()
                            fcur += 1
                    emit_normalize(tci, m, ya, yb)
                for op in endfill:
                    op()

            # ---- epilogue: last chunk's out-projection
            for ch in outproj_chains(NTC - 1):
                for op in ch:
                    op()

    nc.finalize()
    _NC_CACHE["nc"] = nc
    return nc


def _in_maps(x, Wqkv, Wout):
    import ml_dtypes

    bf16 = ml_dtypes.bfloat16
    x = np.ascontiguousarray(np.asarray(x, dtype=np.float32))
    Wqkv = np.ascontiguousarray(np.asarray(Wqkv, dtype=np.float32))
    Wout = np.ascontiguousarray(np.asarray(Wout, dtype=np.float32))
    xTs = [np.ascontiguousarray(x[b].T.astype(bf16)) for b in range(B)]
    maps = []
    for c in range(8):
        b, g = divmod(c, 2)
        qrows = Wqkv[E * g : E * g + E]
        krows = Wqkv[D + E * g : D + E * g + E]
        vrows = Wqkv[2 * D + E * g : 2 * D + E * g + E]
        maps.append(
            {
                "xT": xTs[b],
                "wqkT": np.ascontiguousarray(
                    np.concatenate([qrows, krows], axis=0).T.astype(bf16)
                ),
                "wvT": np.ascontiguousarray(vrows.T.astype(bf16)),
                "woT": np.ascontiguousarray(Wout[:, E * g : E * g + E].T),
            }
        )
    return maps


def _run(x, Wqkv, Wout, trace=False):
    from concourse.bass_utils import run_bass_kernel_spmd

    nc = build()
    res = run_bass_kernel_spmd(
        nc, _in_maps(x, Wqkv, Wout), core_ids=list(range(8)), trace=trace
    )
    out = np.empty((B, T, D), dtype=np.float32)
    for b in range(B):
        out[b] = res.results[2 * b]["z"] + res.results[2 * b + 1]["z"]
    return out, res


def kernel(x, Wqkv, Wout):
    out, _ = _run(x, Wqkv, Wout, trace=False)
    return out


# revision 48
# speedup vs baseline: 1.0738x; 1.0554x over previous
"""Multi-head causal attention on 8 TRN2 NeuronCores.

Problem: x[4,2048,1024] @ Wqkv.T -> 16-head causal attention -> @ Wout.T.

Sharding: core c handles batch b=c//2, head-group g=c%2 (8 heads of 64).
Each core computes qkv for its (batch, head-group) slice, causal attention,
and a partial out-projection over its 512 columns of Wout's input dim.
Host sums the two partials per batch (the all-reduce of the hint).

Per-core layouts (host pre-transposes so every matmul contraction dim lands
on SBUF partitions):
  xT   [1024 d, 2048 t]      wqkT [1024 d, 1024 (q|k)e]
  wvT  [1024 d,  512 e]      woT  [ 512 e, 1024 f]
Q/K/S stay fp32r; P (post-exp) and V are bf16 (err budget 2e-2 allows it).

Schedule: the TRN2 PE clock ramps (1.2 GHz until ~3us of continuous busy,
then 2.4 GHz), so any PE idle gap doubles matmul time.  The attention inner
loop is a depth-3 pipeline over 128-key j-blocks:
    step s:  S(s) matmuls | exp(s-1) on ACT | mask(s-1) on gpsimd | AV(s-3)
so the scalar-engine exp (~1.1us/block vs ~0.85us of PE work) is never on
the PE critical path.  The ACT deficit is covered by interleaving next-chunk
QKV-production and prev-chunk out-projection matmuls as per-step PE filler,
assigned to windows/pairs so tile-pool slot rotation never waits on a
later-emitted reader:
  window tc:  pair0 <- out-proj(tc-1); pair p -> qk/v group p-1 of chunk
  tc+1; group 3 after pair 3.  qk(3) of chunk 3 runs inside window 3
  (pairs 0-2 don't read kt[3]/qt[3]); out-proj(3) is the epilogue.
Diagonal j-blocks only stream/exp query columns >= 128r (clamped to N>=256
for fp32r); affine_select zeroes the masked+stale region afterwards.
"""

import sys

sys.path.insert(0, "/opt/trn_rl_repo")

import numpy as np

B, T, D, H = 4, 2048, 1024, 16
E = 512  # per-core head width (8 heads x 64)
ND = 8  # d chunks of 128
NTC = 4  # t chunks of 512
SCALE = 0.125  # 1/sqrt(64)
Q0R = [0, 128, 256, 256]  # first live query col per diag sub-block r
LAG = 7  # AV trails S by LAG j-blocks: the pair's S-only head
         # steps overlap the previous pair's normalize drain

_NC_CACHE = {}


def build():
    if "nc" in _NC_CACHE:
        return _NC_CACHE["nc"]
    import concourse.bacc as bacc
    import concourse.mybir as mybir
    import concourse.tile as tile

    F32 = mybir.dt.float32
    F32R = mybir.dt.float32r
    BF16 = mybir.dt.bfloat16
    EXP = mybir.ActivationFunctionType.Exp

    nc = bacc.Bacc("TRN2", target_bir_lowering=False, debug=False, num_devices=8)
    xT = nc.declare_dram_parameter("xT", [D, T], BF16, isOutput=False)
    wqkT = nc.declare_dram_parameter("wqkT", [D, 2 * E], BF16, isOutput=False)
    wvT = nc.declare_dram_parameter("wvT", [D, E], BF16, isOutput=False)
    woT = nc.declare_dram_parameter("woT", [E, D], F32R, isOutput=False)
    z = nc.declare_dram_parameter("z", [T, D], F32, isOutput=True)
    dbg = {}
    if _NC_CACHE.get("debug"):
        for nm, shp in [
            ("dpt", [128, 512]), ("dya", [96, 512]), ("drca", [1, 512]),
            ("drba", [64, 512]), ("drbb", [64, 512]), ("dysb", [128, 512]),
            ("dytm", [64, 512]),
        ]:
            dbg[nm] = nc.declare_dram_parameter(nm, shp, F32, isOutput=True)

    with tile.TileContext(nc) as tc_:
        with (
            tc_.tile_pool(name="pw", bufs=8) as pw,
            tc_.tile_pool(name="pwo", bufs=4) as pwo,
            tc_.tile_pool(name="px", bufs=16) as px,
            tc_.tile_pool(name="pkt", bufs=4) as pkt,
            tc_.tile_pool(name="pqt", bufs=4) as pqt,
            tc_.tile_pool(name="pv", bufs=16) as pv,
            tc_.tile_pool(name="ppt", bufs=8) as ppt,
            tc_.tile_pool(name="pr", bufs=2) as pr,
            tc_.tile_pool(name="pysb", bufs=10) as pysb,
            tc_.tile_pool(name="pzsb", bufs=4) as pzsb,
            tc_.tile_pool(name="pst", bufs=2, space="PSUM") as pst,
            tc_.tile_pool(name="pyd", bufs=1, space="PSUM") as pyd,
            tc_.tile_pool(name="pfa", bufs=2, space="PSUM") as pfa,
        ):
            # ---- weights + first x chunks, striped across DMA queues and
            # ordered so the dc=0 QKV chain can start within ~5us: the
            # prologue is input-DMA-bound, so emission order is load order.
            def dma_striped(dst, src, nstripe):
                w = dst.shape[1]
                sw = w // nstripe
                for s_ in range(nstripe):
                    nc.sync.dma_start(
                        dst[:, s_ * sw : (s_ + 1) * sw],
                        src[:, s_ * sw : (s_ + 1) * sw],
                    )

            def emit_x_loads(tci):
                xs = []
                t0 = tci * 512
                for dc in range(ND):
                    t_ = px.tile([128, 512], BF16, tag="x", name="xs")
                    nc.sync.dma_start(
                        t_[:], xT[dc * 128 : (dc + 1) * 128, t0 : t0 + 512]
                    )
                    xs.append(t_)
                return xs

            wqk = []
            xs_p = []
            for dc in range(ND):
                t_ = pw.tile([128, 2 * E], BF16, tag="wqk")
                dma_striped(t_[:], wqkT[dc * 128 : (dc + 1) * 128, :], 4)
                wqk.append(t_)
                tx = px.tile([128, 512], BF16, tag="x", name="xs")
                dma_striped(tx[:], xT[dc * 128 : (dc + 1) * 128, 0:512], 2)
                xs_p.append(tx)
            wv = []
            for dc in range(ND):
                t_ = pw.tile([128, E], BF16, tag="wv")
                nc.sync.dma_start(t_[:], wvT[dc * 128 : (dc + 1) * 128, :])
                wv.append(t_)
            xs_by_tc = {0: xs_p, 1: emit_x_loads(1)}
            wo = []
            for m in range(4):
                t_ = pwo.tile([128, D], F32R, tag="wo")
                nc.sync.dma_start(t_[:], woT[m * 128 : (m + 1) * 128, :])
                wo.append(t_)

            # persistent K^T [e,t] tiles; pair m = heads 2m / 2m+1 at
            # partition rows 0:64 / 64:128
            kt = [
                pkt.tile([128, T], F32R, tag="kt", name=f"kt{i}")
                for i in range(4)
            ]
            vt = [None] * 16  # bf16 [V_h(64)|ones(32)] per head, per j-block
            qt_by_tc = {}  # (tc, m) -> qt tile
            ysb_hist = {}  # (tc, m) -> normalized y^T tile

            # ---- filler chains (lists of zero-arg closures, one PE/DVE op
            # each, executed by the window scheduler's filler cursor)

            def qk_chain(xs, g, tci):
                """g 0..3: Q chunk for pair g; 4..7: K chunk for pair g-4."""
                ops = []
                state = {}

                def mk_mm(dc):
                    def run():
                        if "acc" not in state:
                            state["acc"] = pfa.tile(
                                [128, 512], F32, tag="facc", name="qkacc"
                            )
                        nc.tensor.matmul(
                            state["acc"][:],
                            wqk[dc][:, g * 128 : (g + 1) * 128],
                            xs[dc][:],
                            start=(dc == 0),
                            stop=(dc == ND - 1),
                        )

                    return run

                ops += [mk_mm(dc) for dc in range(ND)]

                def fin():
                    acc = state["acc"]
                    if g < 4:
                        t_ = pqt.tile([128, 512], F32R, tag="qt", name="qt")
                        nc.vector.tensor_copy(t_[:], acc[:])
                        qt_by_tc[(tci, g)] = t_
                    else:
                        t0 = tci * 512
                        nc.vector.tensor_copy(
                            kt[g - 4][:, t0 : t0 + 512], acc[:]
                        )

                ops.append(fin)
                return ops

            def v_chain(xs, tci, ts):
                jb = 4 * tci + ts
                ops = []
                state = {}

                def mk_mm(dc):
                    def run():
                        if "acc" not in state:
                            state["acc"] = pfa.tile(
                                [128, 512], F32, tag="facc", name="vacc"
                            )
                        nc.tensor.matmul(
                            state["acc"][:],
                            xs[dc][:, ts * 128 : (ts + 1) * 128],
                            wv[dc][:],
                            start=(dc == 0),
                            stop=(dc == ND - 1),
                        )

                    return run

                ops += [mk_mm(dc) for dc in range(ND)]

                def fin():
                    acc = state["acc"]
                    t_ = pv.tile([128, 768], BF16, tag="v", name="vt")
                    t4 = t_[:].rearrange("p (hh c) -> p hh c", hh=8)
                    a4 = acc[:].rearrange("p (hh c) -> p hh c", hh=8)
                    nc.vector.tensor_copy(t4[:, :, 0:64], a4[:])
                    nc.vector.memset(t4[:, :, 64:96], 1.0)
                    vt[jb] = t_

                ops.append(fin)
                return ops

            def outproj_chain(tcp, ib, fh):
                ops = []
                state = {}

                def mk_mm(m):
                    def run():
                        if "zp" not in state:
                            state["zp"] = pfa.tile(
                                [128, 512], F32, tag="facc", name="zp"
                            )
                        nc.tensor.matmul(
                            state["zp"][:],
                            ysb_hist[(tcp, m)][:, ib * 128 : (ib + 1) * 128],
                            wo[m][:, fh * 512 : fh * 512 + 512],
                            start=(m == 0),
                            stop=(m == 3),
                        )

                    return run

                ops += [mk_mm(m) for m in range(4)]

                def fin():
                    zsb = pzsb.tile([128, 512], F32, tag="zsb")
                    nc.vector.tensor_copy(zsb[:], state["zp"][:])
                    row = (4 * tcp + ib) * 128
                    ns_ = 4 if tcp == NTC - 1 else 2
                    sw_ = 512 // ns_
                    for s_ in range(ns_):
                        nc.sync.dma_start(
                            z[
                                row : row + 128,
                                fh * 512 + s_ * sw_ : fh * 512 + (s_ + 1) * sw_,
                            ],
                            zsb[:, s_ * sw_ : (s_ + 1) * sw_],
                        )

                ops.append(fin)
                return ops

            def outproj_chains(tcp):
                return [
                    outproj_chain(tcp, ib, fh)
                    for ib in range(4)
                    for fh in range(2)
                ]

            # ---- attention pieces
            def emit_S(tci, m, jb, qtm):
                st = pst.tile([128, 1024], F32, tag="st", name="st")
                r = jb - 4 * tci
                q0 = Q0R[r] if r >= 0 else 0
                for h in range(2):
                    nc.tensor.matmul(
                        st[:, h * 512 + q0 : (h + 1) * 512],
                        kt[m][
                            h * 64 : h * 64 + 64, jb * 128 : (jb + 1) * 128
                        ],
                        qtm[h * 64 : h * 64 + 64, q0:512],
                        start=True,
                        stop=True,
                    )
                return st

            def emit_exp(tci, jb, st):
                pt = ppt.tile([128, 1024], BF16, tag="pt", name="pt")
                r = jb - 4 * tci
                q0 = Q0R[r] if r >= 0 else 0
                if q0 > 0:
                    s3 = st[:].rearrange("p (h q) -> p h q", h=2)
                    p3 = pt[:].rearrange("p (h q) -> p h q", h=2)
                    nc.scalar.activation(
                        p3[:, :, q0:512], s3[:, :, q0:512], EXP, scale=SCALE
                    )
                    # cols < q0 are entirely masked and never exp'd
                    nc.vector.memset(p3[:, :, 0:q0], 0.0)
                else:
                    nc.scalar.activation(pt[:], st[:], EXP, scale=SCALE)
                if r >= 0:
                    # zero masked region in the live cols: keep where
                    # (q' + q0) - k - 128 r >= 0
                    p3 = pt[:].rearrange("p (h q) -> p h q", h=2)
                    nc.gpsimd.affine_select(
                        out=p3[:, :, q0:512],
                        in_=p3[:, :, q0:512],
                        compare_op=mybir.AluOpType.is_ge,
                        fill=0.0,
                        base=q0 - 128 * r,
                        pattern=[[0, 2], [1, 512 - q0]],
                        channel_multiplier=-1,
                    )
                return pt

            def emit_AV(m, jb, pt, ya, yb, first, last):
                nc.tensor.matmul(
                    ya[:],
                    vt[jb][:, m * 192 : m * 192 + 96],
                    pt[:, 0:512],
                    start=first,
                    stop=last,
                )
                nc.tensor.matmul(
                    yb[:],
                    vt[jb][:, m * 192 + 96 : m * 192 + 192],
                    pt[:, 512:1024],
                    start=first,
                    stop=last,
                )

            def emit_normalize(tci, m, ya, yb):
                if dbg and tci == 0 and m == 0:
                    dya_sb = pzsb.tile([128, 512], F32, tag="zsb")
                    nc.vector.tensor_copy(dya_sb[0:96, :], ya[:])
                    nc.sync.dma_start(dbg["dya"][:], dya_sb[0:96, :])
                # ya/yb rows 64:96 hold the softmax denominator (ones
                # columns of vt); reciprocal runs in-place at partition 64,
                # gpsimd broadcasts it down to rows 0:64.
                # critical path after the last AV: copies (DVE+gpsimd in
                # parallel) -> rc0 gather (gen on the idle gpsimd queue so
                # it can't head-of-line-block the sync DMA queue) -> recip
                # -> one wide broadcast -> muls (DVE+gpsimd) -> shuffle
                # (DVE partition move; no DMA anywhere near the hot path)
                rca = pr.tile([128, 1024], F32, tag="rca", bufs=1)
                nc.vector.tensor_copy(rca[64:65, 0:512], ya[64:65, :])
                nc.vector.tensor_copy(rca[64:65, 512:1024], yb[64:65, :])
                # partition-move head B's raw y to rows 64:128 (off the
                # critical path; the mul below does the fp32r rounding)
                ytmp = pr.tile([128, 512], F32, tag="ytmp", bufs=1)
                nc.vector.stream_shuffle(
                    ytmp[64:128, :], yb[0:64, :], mask=list(range(32))
                )
                rc0 = pr.tile([1, 1024], F32, tag="rc0", bufs=1)
                nc.gpsimd.dma_start(rc0[0:1, :], rca[64:65, :])
                rcv = pr.tile([1, 1024], F32, tag="rcv", bufs=1)
                nc.vector.reciprocal_approx_fast(rcv[0:1, :], rc0[0:1, :])
                rb = pr.tile([128, 1024], F32, tag="rb", bufs=1)
                nc.gpsimd.partition_broadcast(rb[:, :], rcv[0:1, :])
                ysb = pysb.tile([128, 512], F32R, tag="ysb", name="ysb")
                nc.vector.tensor_mul(
                    ysb[0:64, :], ya[0:64, :], rb[0:64, 0:512]
                )
                nc.vector.tensor_mul(
                    ysb[64:128, :], ytmp[64:128, :], rb[64:128, 512:1024]
                )
                if dbg and tci == 0 and m == 0:
                    nc.sync.dma_start(dbg["drca"][:], rcv[0:1, 0:512])
                    nc.sync.dma_start(dbg["drba"][:], rb[0:64, 0:512])
                    nc.sync.dma_start(dbg["drbb"][:], rb[64:128, 512:1024])
                    nc.sync.dma_start(dbg["dytm"][:], ytmp[64:128, :])
                    nc.sync.dma_start(dbg["dysb"][:], ysb[:].bitcast(F32))
                ysb_hist[(tci, m)] = ysb

            # ---- prologue: chunk 0 QKV, dense
            xs_cur = xs_by_tc[0]
            for g in range(8):
                for op in qk_chain(xs_cur, g, 0):
                    op()
            for ts in range(4):
                for op in v_chain(xs_cur, 0, ts):
                    op()

            # ---- windows
            for tci in range(NTC):
                njb = 4 * tci + 4
                pairfill = [[], [], [], []]
                endfill = []
                if tci + 2 < NTC:
                    xs_by_tc[tci + 2] = emit_x_loads(tci + 2)
                if tci == 1:
                    for i, ch in enumerate(outproj_chains(0)):
                        pairfill[i % 4] += ch
                if tci + 1 < NTC:
                    xs_next = xs_by_tc[tci + 1]
                    if tci < 2:
                        for g in range(3):
                            pairfill[g + 1] += qk_chain(xs_next, g, tci + 1)
                            pairfill[g + 1] += qk_chain(
                                xs_next, g + 4, tci + 1
                            )
                            pairfill[g + 1] += v_chain(xs_next, tci + 1, g)
                        endfill += qk_chain(xs_next, 3, tci + 1)
                        endfill += qk_chain(xs_next, 7, tci + 1)
                        endfill += v_chain(xs_next, tci + 1, 3)
                    else:
                        # window 2: chunk 3's qkv minus its pair-3 qk, which
                        # runs inside window 3 (whose pairs 0-2 don't touch
                        # kt[3]/qt[3]); out-proj(1) and (2) also defer to
                        # window 3, whose ACT-paced steps need PE filler
                        pairfill[0] += v_chain(xs_next, tci + 1, 0)
                        pairfill[0] += v_chain(xs_next, tci + 1, 1)
                        for g in range(3):
                            pairfill[g + 1] += qk_chain(xs_next, g, tci + 1)
                            pairfill[g + 1] += qk_chain(
                                xs_next, g + 4, tci + 1
                            )
                        pairfill[3] += v_chain(xs_next, tci + 1, 2)
                        endfill += v_chain(xs_next, tci + 1, 3)
                        _w3qk = qk_chain(xs_next, 3, tci + 1) + qk_chain(
                            xs_next, 7, tci + 1
                        )
                else:
                    op1 = outproj_chains(1)
                    op2 = outproj_chains(2)
                    pairfill[0] = _w3qk + op1[0] + op1[1]
                    pairfill[1] = op1[2] + op1[3] + op1[4] + op1[5]
                    pairfill[2] = op1[6] + op1[7] + op2[0] + op2[1] + op2[2]
                    pairfill[3] = op2[3] + op2[4] + op2[5] + op2[6] + op2[7]

                for m in range(4):
                    qtm = qt_by_tc[(tci, m)]
                    ya = pyd.tile([96, 512], F32, tag="ya", bufs=1)
                    yb = pyd.tile([96, 512], F32, tag="yb", bufs=1)
                    fl = pairfill[m]
                    fcur = 0
                    nsteps = njb + LAG
                    ring_st = [None] * 2
                    ring_pt = [None] * 8
                    for s in range(nsteps):
                        if s < njb:
                            ring_st[s % 2] = emit_S(tci, m, s, qtm)
                        if 1 <= s <= njb:
                            jb = s - 1
                            ring_pt[jb % 8] = emit_exp(
                                tci, jb, ring_st[jb % 2]
                            )
                            if dbg and tci == 0 and m == 0 and jb == 0:
                                nc.sync.dma_start(
                                    dbg["dpt"][:],
                                    ring_pt[0][:].bitcast(F32),
                                )
                        if LAG <= s < njb + LAG:
                            jb = s - LAG
                            emit_AV(
                                m,
                                jb,
                                ring_pt[jb % 8],
                                ya,
                                yb,
                                jb == 0,
                                jb == njb - 1,
                            )
                        target = len(fl) * (s + 1) // nsteps
                        while fcur < target:
                            fl[fcur]()
                            fcur += 1
                    emit_normalize(tci, m, ya, yb)
                for op in endfill:
                    op()

            # ---- epilogue: last chunk's out-projection
            for ch in outproj_chains(NTC - 1):
                for op in ch:
                    op()

    nc.finalize()
    _NC_CACHE["nc"] = nc
    return nc


def _in_maps(x, Wqkv, Wout):
    import ml_dtypes

    bf16 = ml_dtypes.bfloat16
    x = np.ascontiguousarray(np.asarray(x, dtype=np.float32))
    Wqkv = np.ascontiguousarray(np.asarray(Wqkv, dtype=np.float32))
    Wout = np.ascontiguousarray(np.asarray(Wout, dtype=np.float32))
    xTs = [np.ascontiguousarray(x[b].T.astype(bf16)) for b in range(B)]
    maps = []
    for c in range(8):
        b, g = divmod(c, 2)
        qrows = Wqkv[E * g : E * g + E]
        krows = Wqkv[D + E * g : D + E * g + E]
        vrows = Wqkv[2 * D + E * g : 2 * D + E * g + E]
        maps.append(
            {
                "xT": xTs[b],
                "wqkT": np.ascontiguousarray(
                    np.concatenate([qrows, krows], axis=0).T.astype(bf16)
                ),
                "wvT": np.ascontiguousarray(vrows.T.astype(bf16)),
                "woT": np.ascontiguousarray(Wout[:, E * g : E * g + E].T),
            }
        )
    return maps


def _run(x, Wqkv, Wout, trace=False):
    from concourse.bass_utils import run_bass_kernel_spmd

    nc = build()
    res = run_bass_kernel_spmd(
        nc, _in_maps(x, Wqkv, Wout), core_ids=list(range(8)), trace=trace
    )
    out = np.empty((B, T, D), dtype=np.float32)
    for b in range(B):
        out[b] = res.results[2 * b]["z"] + res.results[2 * b + 1]["z"]
    return out, res


def kernel(x, Wqkv, Wout):
    out, _ = _run(x, Wqkv, Wout, trace=False)
    return out
